# revision 2
# baseline (speedup 1.0000x reference)
"""RGCN message-passing kernel for Trainium2 (8 NeuronCores).

Graph-parallel sharding: 2000 subgraphs -> 250 per core. Each subgraph's
100 nodes and ~1000 edges stay on one core; the tiny RGCN/MLP weights are
replicated. The edge list is converted on the host into per-graph
mean-normalized dense adjacency blocks (standard GNN dataloader-style
preprocessing); all layer compute (aggregate, transform, activations,
readout MLP) runs on device via PE matmuls.
"""

import os
import sys
import types
import contextlib
import ctypes

import numpy as np
import ml_dtypes

import concourse.bass as bass
import concourse.mybir as mybir
import concourse.tile as tile_mod
from concourse.tile import TileContext
from concourse import bass_utils

BF16 = ml_dtypes.bfloat16

# Problem constants
N = 200_000
G = 2_000
NPG = 100
R = 3
F_IN = 4
F = 32          # padded feature width, uniform across layers
L = 4           # layers
NCORES = 8
GC = G // NCORES          # graphs per core (250)
CH = 10                   # graphs per chunk
NCHUNK = GC // CH         # 25

# ---------------------------------------------------------------------------
# Workaround: this container's walrus rejects >1 sem-wait per instruction.
# Split extra waits onto same-engine no-ops right after Tile finishes.
# ---------------------------------------------------------------------------
_orig_drain_and_barrier = tile_mod.TileContext._drain_and_barrier


def _split_multiwaits(nc):
    for f in nc.m.functions:
        for blk in f.blocks:
            insts = blk.instructions
            new = []
            changed = False
            for ins in insts:
                si = ins.sync_info
                if si is not None and len(si.on_wait) > 1:
                    ow = list(si.on_wait)
                    for i, w in enumerate(ow[:-1]):
                        new.append(
                            mybir.InstNoOp(
                                name=f"{ins.name}-sw{i}",
                                engine=ins.engine,
                                bass_nofuse=True,
                                sync_info=mybir.SyncInfo(on_wait=[w], on_update=[]),
                            )
                        )
                    ins.sync_info = mybir.SyncInfo(
                        on_wait=[ow[-1]], on_update=list(si.on_update)
                    )
                    changed = True
                new.append(ins)
            if changed:
                blk.instructions = new


def _patched_drain_and_barrier(self, tick_clock, wait_clock):
    _orig_drain_and_barrier(self, tick_clock, wait_clock)
    _split_multiwaits(self.nc)


tile_mod.TileContext._drain_and_barrier = _patched_drain_and_barrier


# ---------------------------------------------------------------------------
# Optional NTFF profile hook (lets BASS_TRACE=1 produce HW exec time under
# axon). Safe no-op if the .so is missing.
# ---------------------------------------------------------------------------
def _install_ntff_hook():
    so_path = "/opt/axon/libaxon_pjrt.so"
    if "antenv.axon_hooks" in sys.modules or not os.path.exists(so_path):
        return
    try:
        lib = ctypes.CDLL(so_path)
        if not hasattr(lib, "axon_start_nrt_profile"):
            return
        lib.axon_start_nrt_profile.argtypes = [
            ctypes.POINTER(ctypes.c_int64),
            ctypes.c_size_t,
        ]
        lib.axon_start_nrt_profile.restype = ctypes.c_int64
        lib.axon_stop_nrt_profile.argtypes = [ctypes.c_char_p]
        lib.axon_stop_nrt_profile.restype = ctypes.c_int64

        @contextlib.contextmanager
        def hook(output_dir, device_ids=None):
            import jax

            jax.devices()
            if device_ids:
                ids = (ctypes.c_int64 * len(device_ids))(*device_ids)
                rc = lib.axon_start_nrt_profile(ids, len(device_ids))
            else:
                rc = lib.axon_start_nrt_profile(None, 0)
            if rc != 0:
                raise RuntimeError(f"axon_start_nrt_profile rc={rc}")
            try:
                yield
            finally:
                n = lib.axon_stop_nrt_profile(str(output_dir).encode())
                if n <= 0:
                    print(f"profile: {n} files written", file=sys.stderr)

        mod = types.ModuleType("antenv.axon_hooks")
        mod.set_axon_ntff_profile_hook = lambda h: None
        mod.get_axon_ntff_profile_hook = lambda: hook
        sys.modules["antenv.axon_hooks"] = mod
        import antenv

        antenv.axon_hooks = mod
    except Exception:
        pass


_install_ntff_hook()


# ---------------------------------------------------------------------------
# Host-side sharding / layout prep
# ---------------------------------------------------------------------------
def _prep_core_inputs(x, edge_index, edge_type, basis, comp, root, bias, w1, b1, w2, b2):
    x = np.asarray(x, np.float32)
    src = np.asarray(edge_index[0], np.int64)
    dst = np.asarray(edge_index[1], np.int64)
    et = np.asarray(edge_type, np.int64)

    g = src // NPG
    assert (dst // NPG == g).all(), "edges must stay within a subgraph"

    # bundle/user node positions: reference marks x[:,0]==1 (bundle) and
    # x[:,1]==1 (user). By construction they are local rows 0 and 1; if not,
    # permute nodes within each graph so they are.
    xr = x.reshape(G, NPG, F_IN)
    std = (xr[:, 0, 0] == 1.0).all() and (xr[:, 1, 1] == 1.0).all()
    if not std:
        bpos = np.argmax(xr[:, :, 0] == 1.0, axis=1)
        upos = np.argmax(xr[:, :, 1] == 1.0, axis=1)
        perm = np.tile(np.arange(NPG), (G, 1))
        for gi in range(G):
            order = [bpos[gi], upos[gi]] + [
                p for p in range(NPG) if p not in (bpos[gi], upos[gi])
            ]
            perm[gi] = np.array(order)
        inv = np.argsort(perm, axis=1)
        xr = np.take_along_axis(xr, perm[:, :, None], axis=1)
        src = g * NPG + inv[g, src % NPG]
        dst = (dst // NPG) * NPG + inv[dst // NPG, dst % NPG]
        x = xr.reshape(N, F_IN)

    # Layer weights: W_cat[l] rows 0..95 = relation transforms, 96..127 root.
    wcat = np.zeros((L, 4 * F, F), np.float32)
    for li in range(L):
        W = np.einsum("rb,bio->rio", np.asarray(comp[li], np.float32),
                      np.asarray(basis[li], np.float32))
        din = W.shape[1]
        for r in range(R):
            wcat[li, 32 * r : 32 * r + din] = W[r]
        wcat[li, 96 : 96 + din] = np.asarray(root[li], np.float32)
    wcat = wcat.astype(BF16)

    lbias = np.stack([np.asarray(b, np.float32) for b in bias])  # [L, 32]
    assert np.abs(lbias).max() == 0.0, "nonzero RGCN bias not supported"

    w1 = np.asarray(w1, np.float32)
    b1 = np.asarray(b1, np.float32)
    w2 = np.asarray(w2, np.float32)
    b2 = np.asarray(b2, np.float32).reshape(1)
    w1u = w1[:128].astype(BF16)
    w1b = w1[128:].astype(BF16)
    b1r = b1.reshape(1, 128).astype(BF16)
    w2c = w2.reshape(128, 1).astype(BF16)
    b2r = b2.reshape(1, 1).astype(BF16)
    ones = np.ones((1, GC), BF16)
    eye = np.eye(NPG, dtype=np.float32)
    i100 = eye.astype(BF16)
    sel = eye[:, :2].astype(BF16)

    in_maps = []
    for c in range(NCORES):
        glo, ghi = c * GC, (c + 1) * GC
        emask = (g >= glo) & (g < ghi)
        gl = g[emask] - glo
        sl = src[emask] % NPG
        dl = dst[emask] % NPG
        rl = et[emask]
        flat = ((gl * R + rl) * NPG + dl) * NPG + sl
        counts = np.bincount(flat, minlength=GC * R * NPG * NPG).astype(np.float32)
        counts = counts.reshape(GC, R, NPG, NPG)
        cnt = counts.sum(axis=3, keepdims=True)
        Bn = counts / np.maximum(cnt, 1.0)
        # -> [src, (graph, relation, dst)]
        Bt = np.ascontiguousarray(Bn.transpose(3, 0, 1, 2)).reshape(
            NPG, GC * R * NPG
        ).astype(BF16)

        xs = x[glo * NPG : ghi * NPG].reshape(GC, NPG, F_IN)
        Xp = np.zeros((NPG, GC, F), np.float32)
        Xp[:, :, :F_IN] = xs.transpose(1, 0, 2)
        Xp = Xp.reshape(NPG, GC * F).astype(BF16)

        in_maps.append(
            dict(
                B=Bt, X=Xp, WCAT=wcat, W1U=w1u, W1B=w1b, B1R=b1r, W2C=w2c,
                B2R=b2r, ONES=ones, I100=i100, SEL=sel,
            )
        )
    return in_maps


# ---------------------------------------------------------------------------
# Device kernel
# ---------------------------------------------------------------------------
def _build_nc():
    f32 = mybir.dt.float32
    bf = mybir.dt.bfloat16
    nc = bass.Bass("TRN2")

    Bd = nc.dram_tensor("B", [NPG, GC * R * NPG], bf, kind="ExternalInput")
    Xd = nc.dram_tensor("X", [NPG, GC * F], bf, kind="ExternalInput")
    Wd = nc.dram_tensor("WCAT", [L, 4 * F, F], bf, kind="ExternalInput")
    W1Ud = nc.dram_tensor("W1U", [128, 128], bf, kind="ExternalInput")
    W1Bd = nc.dram_tensor("W1B", [128, 128], bf, kind="ExternalInput")
    B1Rd = nc.dram_tensor("B1R", [1, 128], bf, kind="ExternalInput")
    W2Cd = nc.dram_tensor("W2C", [128, 1], bf, kind="ExternalInput")
    B2Rd = nc.dram_tensor("B2R", [1, 1], bf, kind="ExternalInput")
    ONESd = nc.dram_tensor("ONES", [1, GC], bf, kind="ExternalInput")
    I100d = nc.dram_tensor("I100", [NPG, NPG], bf, kind="ExternalInput")
    SELd = nc.dram_tensor("SEL", [NPG, 2], bf, kind="ExternalInput")
    OUTd = nc.dram_tensor("OUT", [1, GC], f32, kind="ExternalOutput")

    with TileContext(nc) as tc:
        with (
            tc.tile_pool(name="bres", bufs=1) as bresp,
            tc.tile_pool(name="hbuf", bufs=1) as hp,
            tc.tile_pool(name="consts", bufs=1) as cp,
            tc.tile_pool(name="ssb", bufs=3) as ssbp,
            tc.tile_pool(name="spsum", bufs=2, space="PSUM") as spp,
            tc.tile_pool(name="bpsum", bufs=2, space="PSUM") as bpp,
            tc.tile_pool(name="tailpsum", bufs=2, space="PSUM") as tailp,
        ):
            Bres = bresp.tile([128, GC * R * NPG], bf)
            nslc = 5
            w = GC * R * NPG // nslc
            for i in range(nslc):
                nc.sync.dma_start(
                    out=Bres[0:NPG, i * w : (i + 1) * w],
                    in_=Bd[:, i * w : (i + 1) * w],
                )

            hA = hp.tile([128, GC * F], bf, tag="hA")
            nc.sync.dma_start(out=hA[0:NPG, :], in_=Xd[:, :])

            Wcat = cp.tile([128, L * F], bf, tag="wcat")
            for li in range(L):
                nc.sync.dma_start(
                    out=Wcat[:, li * F : (li + 1) * F], in_=Wd[li, :, :]
                )
            I100 = cp.tile([128, NPG], bf, tag="i100")
            nc.sync.dma_start(out=I100[0:NPG, :], in_=I100d[:, :])
            SEL = cp.tile([128, 2], bf, tag="sel")
            nc.sync.dma_start(out=SEL[0:NPG, :], in_=SELd[:, :])
            W1U = cp.tile([128, 128], bf, tag="w1u")
            nc.sync.dma_start(out=W1U[:, :], in_=W1Ud[:, :])
            W1B = cp.tile([128, 128], bf, tag="w1b")
            nc.sync.dma_start(out=W1B[:, :], in_=W1Bd[:, :])
            B1R = cp.tile([1, 128], bf, tag="b1r")
            nc.sync.dma_start(out=B1R[:, :], in_=B1Rd[:, :])
            W2C = cp.tile([128, 1], bf, tag="w2c")
            nc.sync.dma_start(out=W2C[:, :], in_=W2Cd[:, :])
            B2R = cp.tile([1, 1], bf, tag="b2r")
            nc.sync.dma_start(out=B2R[:, :], in_=B2Rd[:, :])
            ONES = cp.tile([1, GC], bf, tag="ones")
            nc.sync.dma_start(out=ONES[:, :], in_=ONESd[:, :])
            Fu = cp.tile([128, GC], bf, tag="fu")
            Fb = cp.tile([128, GC], bf, tag="fb")

            hcur = hA
            for li in range(L):
                hnext = hp.tile([128, GC * F], bf, tag=("hB" if li % 2 == 0 else "hA"))
                for k in range(NCHUNK):
                    Sp = spp.tile([128, CH * NPG], mybir.dt.float32)
                    for j in range(CH):
                        gg = k * CH + j
                        lh = hcur[0:NPG, gg * F : (gg + 1) * F]
                        for r in range(R):
                            nc.tensor.matmul(
                                out=Sp[32 * r : 32 * r + 32, j * NPG : (j + 1) * NPG],
                                lhsT=lh,
                                rhs=Bres[
                                    0:NPG,
                                    (gg * R + r) * NPG : (gg * R + r + 1) * NPG,
                                ],
                                start=True,
                                stop=True,
                            )
                        nc.tensor.matmul(
                            out=Sp[96:128, j * NPG : (j + 1) * NPG],
                            lhsT=lh,
                            rhs=I100[0:NPG, :],
                            start=True,
                            stop=True,
                            tile_position=(0, 96),
                        )
                    Ssb = ssbp.tile([128, CH * NPG], bf)
                    if k % 2 == 0:
                        nc.vector.tensor_copy(out=Ssb[:, :], in_=Sp[:, :])
                    else:
                        nc.scalar.copy(out=Ssb[:, :], in_=Sp[:, :])
                    Bp = bpp.tile([128, CH * F], mybir.dt.float32)
                    for j in range(CH):
                        nc.tensor.matmul(
                            out=Bp[0:NPG, j * F : (j + 1) * F],
                            lhsT=Ssb[:, j * NPG : (j + 1) * NPG],
                            rhs=Wcat[:, li * F : (li + 1) * F],
                            start=True,
                            stop=True,
                        )
                    nc.scalar.activation(
                        out=hnext[0:NPG, k * CH * F : (k + 1) * CH * F],
                        in_=Bp[0:NPG, :],
                        func=mybir.ActivationFunctionType.Tanh,
                    )
                # readout extraction of h_{li+1}
                Hp = tailp.tile([32, 2 * GC], mybir.dt.float32, tag="tail")
                for gg in range(GC):
                    nc.tensor.matmul(
                        out=Hp[:, 2 * gg : 2 * gg + 2],
                        lhsT=hnext[0:NPG, gg * F : (gg + 1) * F],
                        rhs=SEL[0:NPG, :],
                        start=True,
                        stop=True,
                    )
                nc.vector.tensor_copy(
                    out=Fb[32 * li : 32 * li + 32, :], in_=Hp[:, 0::2]
                )
                nc.vector.tensor_copy(
                    out=Fu[32 * li : 32 * li + 32, :], in_=Hp[:, 1::2]
                )
                hcur = hnext

            # MLP head
            Zp = tailp.tile([128, GC], mybir.dt.float32, tag="tail")
            nc.tensor.matmul(out=Zp[:, :], lhsT=W1U[:, :], rhs=Fu[:, :],
                             start=True, stop=False)
            nc.tensor.matmul(out=Zp[:, :], lhsT=W1B[:, :], rhs=Fb[:, :],
                             start=False, stop=False)
            nc.tensor.matmul(out=Zp[:, :], lhsT=B1R[:, :], rhs=ONES[:, :],
                             start=False, stop=True)
            Z1 = cp.tile([128, GC], bf, tag="z1s")
            nc.scalar.activation(out=Z1[:, :], in_=Zp[:, :],
                                 func=mybir.ActivationFunctionType.Relu)
            Z2p = tailp.tile([32, GC], mybir.dt.float32, tag="tail")
            nc.tensor.matmul(out=Z2p[0:1, :], lhsT=W2C[:, :], rhs=Z1[:, :],
                             start=True, stop=False)
            nc.tensor.matmul(out=Z2p[0:1, :], lhsT=B2R[:, :], rhs=ONES[:, :],
                             start=False, stop=True)
            OutS = cp.tile([1, GC], mybir.dt.float32, tag="outs")
            nc.scalar.activation(out=OutS[:, :], in_=Z2p[0:1, :],
                                 func=mybir.ActivationFunctionType.Sigmoid)
            nc.sync.dma_start(out=OUTd[:, :], in_=OutS[:, :])

    return nc


_NC_CACHE = None


def kernel(x, edge_index, edge_type, num_graphs, basis, comp, root, bias,
           w1, b1, w2, b2):
    global _NC_CACHE
    in_maps = _prep_core_inputs(x, edge_index, edge_type, basis, comp, root,
                                bias, w1, b1, w2, b2)
    if _NC_CACHE is None:
        _NC_CACHE = _build_nc()
    nc = _NC_CACHE
    res = bass_utils.run_bass_kernel_spmd(nc, in_maps, core_ids=list(range(NCORES)))
    out = np.concatenate([res.results[c]["OUT"][0] for c in range(NCORES)])
    kernel._last_results = res
    return out.astype(np.float32)
